# revision 72
# baseline (speedup 1.0000x reference)
"""LayerNorm-LSTMCell Bass kernel for Trainium2, data-parallel over batch on 8 NeuronCores.

Computes, per the reference nn.Module:
    gates = x @ W_i + h_prev @ W_h + b          # [B, 4H], gate order i|f|g|o
    i, f, g, o = split(gates);  i,f,o = sigmoid; g = tanh
    c = f * c_prev + i * g
    h = LayerNorm(o * tanh(c)) * ln_weight + ln_bias
Returns (h, c), both [B, H] fp32.

Sharding: batch B=16384 split 8 ways (2048 rows/core); weights replicated.

Matmul path: fp8e4m3 DoubleRow with per-gate error compensation. Host-side,
z=[x|h_prev] is scaled by 8 and split into hi = fp8(8z), lo = fp8(8z - hi);
W=[W_i;W_h] is scaled by 32 and split the same way. Per gate, the device
accumulates GATE_TERMS DoubleRow passes (all sharing the 256x scale) in one
PSUM group: the sigmoid gates i/f use pure fp8 (hi@Wh only — their slope
<= 1/4 damps the quantization noise), while g (tanh, feeds c directly) and o
(scales h) get full 3-term compensation hi@Wh + lo@Wh + hi@Wl. DoubleRow
contracts K=256 per matmul ([128, 2, free] APs over adjacent k-block pairs).
The 1/256 descale and the per-gate-constant bias (b_f=1, rest 0 for this
module) both ride the ACT activation's scale/bias operands for free — no bias
matmuls or PSUM adds on the device at all (a general-bias fallback path keeps
arbitrary b correct). End-to-end rel err 1.48e-2 on HW vs the 2e-2 gate
(3-term-everywhere: 4.3e-3; bf16 matmul baseline: 4.7e-3).

Per-core device schedule (TimelineSim 71.1us vs 124.8us for the bf16 version):
  - Gates accumulate in two bank-pair PSUM tiles [i|f] and [o|g] (2+2 banks,
    2 tiles in flight); per 128-row batch tile: per-gate terms x 4 DoubleRow
    k-blocks x 512-col W slices (32 matmuls). Each bank's matmuls form one
    serial PSUM accumulation chain, so DMA arrival order is matched exactly
    to chain order: wh k-pairs, zh/zl quad 0, wl k-pairs (o|g half only — i/f
    never read Wl), c, then steady zhl/c quads (zh and zl are stacked in one
    dram tensor: one DMA per quad, full-width 512B runs keep the DMA model at
    full rate).
  - K=1 dummy matmuls bridge the DMA-bound head so the PE p-state ramp is
    warm when real data arrives; a tiny early sigmoid hoists the act-table
    load into the head.
  - Epilogue per tile: sigmoid f/i (split only when their constant biases
    differ), tanh g, sigmoid o (scale=1/256, bias fused), c = f*c_prev + i*g
    across Pool+DVE, then tanh(c) + LN one tile later (the "back" half) so
    the in-order ACT queue never stalls on the c round-trip; LN stats via
    bn_stats/bn_aggr, rsqrt(var+eps) by bit-trick seed + 1 Newton step on
    DVE, scale/shift fused in one tensor_scalar. Elementwise tiles bf16.
  - Last two tiles interleave their epilogues; stores batched per quad,
    per-tile for the last quad (shorter tail).
  - All DMAs are HWDGE on the SP queue (single queue = exact arrival order).
"""

import os

import numpy as np

N_CORES = 8
B, I_DIM, H = 16384, 512, 512
G4 = 4 * H          # 2048 gate columns
BS = B // N_CORES   # 2048 batch rows per core
P = 128
NT = BS // P        # 16 batch tiles per core
QUAD = 4            # batch tiles batched per load/store DMA
KB = (I_DIM + H) // P  # 8 contraction k-blocks
DRB = KB // 2       # 4 DoubleRow k-blocks (256 rows each)
# fp8 correction terms per gate (0=i,1=f,2=o,3=g): term 0 = hi@Wh, 1 = lo@Wh,
# 2 = hi@Wl. The sigmoid gates i/f tolerate pure-fp8 (slope <= 1/4 and c/h
# depend on them weakly); g (tanh, feeds c directly) and o (scales h) keep
# full 3-term compensation. Measured end-to-end rel err 1.4e-2 vs the 2e-2
# gate (3-term everywhere: 2.2e-3).
GATE_TERMS = {0: (0,), 1: (0,), 2: (0, 1, 2), 3: (0, 1, 2)}
LN_EPS = 1e-5
RSQRT_MAGIC = 0x5F3759DF
SZ = 8.0            # z pre-scale before fp8 split
SW = 32.0           # W pre-scale before fp8 split
SG = SZ * SW        # PSUM gate scale (descaled in ACT reads)
OUTQ_BUFS = 2
EPI_BUFS = 3
STAT_BUFS = 3

_CACHE = {}


def _newton_inv(nc, mybir, stat_pool, magic, mv, tagp):
    """1/sqrt(var+eps) via bit-trick seed + 1 Newton step; also -mu*inv."""
    F32, I32 = mybir.dt.float32, mybir.dt.int32
    OP = mybir.AluOpType
    v_g = stat_pool.tile([P, 1], F32, tag=tagp + "v")
    nc.vector.tensor_scalar_add(v_g[:], mv[:, 1:2], LN_EPS)
    inv = stat_pool.tile([P, 1], F32, tag=tagp + "i")
    y_i = inv.bitcast(I32)
    nc.vector.tensor_scalar(y_i[:], v_g[:].bitcast(I32), 1, None,
                            op0=OP.logical_shift_right)
    nc.vector.tensor_sub(y_i[:], magic[:], y_i[:])
    nt1 = stat_pool.tile([P, 1], F32, tag=tagp + "n")
    for _ in range(1):  # Newton: y = y * (1.5 - 0.5 * v * y^2)
        nc.vector.tensor_mul(nt1[:], inv[:], inv[:])
        nc.vector.tensor_mul(nt1[:], nt1[:], v_g[:])
        nc.vector.tensor_scalar(nt1[:], nt1[:], -0.5, 1.5,
                                op0=OP.mult, op1=OP.add)
        nc.vector.tensor_mul(inv[:], inv[:], nt1[:])
    nms = stat_pool.tile([P, 1], F32, tag=tagp + "m")
    nc.vector.scalar_tensor_tensor(nms[:], mv[:, 0:1], -1.0, inv[:],
                                   op0=OP.mult, op1=OP.mult)
    return inv, nms


def _emit(nc, tc, ctx, bias_const):
    """bias_const: 4-tuple of per-gate constant bias (i,f,o,g order) when each
    gate's bias is uniform across its columns — it then rides the ACT
    activation's scalar bias operand for free. None -> general path (bias via
    PE K=1 seeds + DVE PSUM adds)."""
    import concourse.bass as bass
    import concourse.mybir as mybir

    F32, BF16, I32 = mybir.dt.float32, mybir.dt.bfloat16, mybir.dt.int32
    F8 = mybir.dt.float8e4
    AF = mybir.ActivationFunctionType
    OP = mybir.AluOpType
    DR = mybir.MatmulPerfMode.DoubleRow

    # zh stacked atop zl in one tensor: quad loads are a single DMA each
    zhl_d = nc.dram_tensor("zhl", [2 * KB * P, BS], F8, kind="ExternalInput").ap()
    wh_d = nc.dram_tensor("wh", [KB * P, G4], F8, kind="ExternalInput").ap()
    wl_d = nc.dram_tensor("wl", [KB * P, 2 * H], F8, kind="ExternalInput").ap()
    c_d = nc.dram_tensor("c_prev", [BS, H], BF16, kind="ExternalInput").ap()
    b16_d = (None if bias_const is not None else
             nc.dram_tensor("b16", [G4], BF16, kind="ExternalInput").ap())
    ho_d = nc.dram_tensor("h_out", [BS, H], BF16, kind="ExternalOutput").ap()
    co_d = nc.dram_tensor("c_out", [BS, H], BF16, kind="ExternalOutput").ap()

    consts = ctx.enter_context(tc.tile_pool(name="consts", bufs=1))
    loads = ctx.enter_context(tc.tile_pool(name="loads", bufs=1))
    outq = ctx.enter_context(tc.tile_pool(name="outq", bufs=OUTQ_BUFS))
    epi = ctx.enter_context(tc.tile_pool(name="epi", bufs=EPI_BUFS))
    stat_pool = ctx.enter_context(tc.tile_pool(name="stats", bufs=STAT_BUFS))
    psum_g = ctx.enter_context(tc.tile_pool(name="psum_g", bufs=2, space="PSUM"))

    # --- staged loads, two HWDGE queues ---------------------------------------
    # ACT queue (idle in the load phase): b16, wh k-pairs, b_bc.
    # SP queue: z quad-0 pieces, c quad-0, wl k-pairs, then zhl/c quads.
    # Tiles 0/1 interleave their matmul terms (see emit loop) so t1's
    # term-1/2 work covers the wl arrival latency.
    wh_sb = consts.tile([P, KB, G4], F8)
    wl_sb = consts.tile([P, KB, 2 * H], F8)
    zhl_sb = consts.tile([P, 2 * KB, BS], F8)
    ones_bf = consts.tile([1, P], BF16)
    nc.vector.memset(ones_bf, 1.0)
    warm = consts.tile([1, H], BF16)
    nc.vector.memset(warm, 0.0)
    # Tiny dummy activation: hoists the sigmoid/tanh act-table load into the
    # DMA-bound head (its 1283ns runs on ACT.ENGINE concurrent to DMA gens).
    act_warm = consts.tile([1, 1], BF16)
    nc.scalar.activation(act_warm[:], warm[0:1, 0:1],
                         mybir.ActivationFunctionType.Sigmoid)
    # Single SP queue, exact consumption order: DMA transfers serialize on one
    # DMA_ENGINES device, so arrival order == issue order; DGE gen (625ns per
    # dma_start) stays well ahead of the ~1.4us transfers.
    def w_pair(wsb, wd, k):
        nc.sync.dma_start(out=wsb[:, k:k + 2, :],
                          in_=wd[k * P:(k + 2) * P, :].rearrange(
                              "(n p) d -> p n d", p=P))

    def zhl_half(half, c0, c1):
        """Load one hi/lo half of zhl dram cols [c0:c1) (full-width runs)."""
        r0 = half * KB
        nc.sync.dma_start(
            out=zhl_sb[:, r0:r0 + KB, c0:c1],
            in_=zhl_d[r0 * P:(r0 + KB) * P, c0:c1].rearrange(
                "(n p) d -> p n d", p=P))

    def dram_quad(ap2d, q):
        return ap2d[q * QUAD * P:(q + 1) * QUAD * P, :].rearrange(
            "(n p) d -> p n d", p=P)

    c_all = loads.tile([P, NT, H], BF16)
    if bias_const is None:
        b_bf = consts.tile([1, G4], BF16)
        nc.sync.dma_start(out=b_bf[:], in_=bass.AP(
            tensor=b16_d.tensor, offset=b16_d.offset, ap=[[0, 1], [1, G4]]))
        b_bc = consts.tile([P, G4], BF16)
    # Arrival order matches the per-bank accumulation chains' consumption
    # order exactly: wh k-pairs (term 1), zl (term 2), wl k-pairs (term 3).
    w_pair(wh_sb, wh_d, 0)
    zhl_half(0, 0, QUAD * P)          # zh quad 0 (512B runs: full DMA rate)
    for k in range(2, KB, 2):
        w_pair(wh_sb, wh_d, k)
    zhl_half(1, 0, QUAD * P)          # zl quad 0
    for k in range(0, KB, 2):
        w_pair(wl_sb, wl_d, k)
    nc.sync.dma_start(out=c_all[:, 0:QUAD, :], in_=dram_quad(c_d, 0))
    if bias_const is None:
        nc.sync.dma_start(out=b_bc[:], in_=bass.AP(
            tensor=b16_d.tensor, offset=b16_d.offset, ap=[[0, P], [1, G4]]))
    # steady quads: one full zhl DMA per quad, c quads trailing
    for q in range(1, NT // QUAD):
        nc.sync.dma_start(
            out=zhl_sb[:, :, q * QUAD * P:(q + 1) * QUAD * P],
            in_=zhl_d[:, q * QUAD * P:(q + 1) * QUAD * P].rearrange(
                "(n p) d -> p n d", p=P))
        nc.sync.dma_start(out=c_all[:, q * QUAD:(q + 1) * QUAD, :],
                          in_=dram_quad(c_d, q))

    magic = consts.tile([P, 1], I32)
    nc.vector.memset(magic, RSQRT_MAGIC)

    # --- main loop -----------------------------------------------------------
    out_tiles = {}
    # term = (z k-block offset into zhl_sb, W tile): hi@Wh, lo@Wh, hi@Wl
    TERMS = [(0, wh_sb), (KB, wh_sb), (0, wl_sb)]

    def w_slice(wsb, j, g0):
        # wl_sb only holds the o|g gate half (i/f never read it)
        c0 = (g0 - 2) * H if wsb is wl_sb else g0 * H
        return wsb[:, 2 * j:2 * j + 2, c0:c0 + H]

    def alloc_tile(t):
        q, tq = divmod(t, QUAD)
        if tq == 0:
            c4_sb = outq.tile([P, QUAD, H], BF16, tag="c4_sb")
            h4_sb = outq.tile([P, QUAD, H], BF16, tag="h4_sb")
            out_tiles[q] = (c4_sb, h4_sb)
        G_if = psum_g.tile([P, 2 * H], F32, tag="Gif")
        G_og = psum_g.tile([P, 2 * H], F32, tag="Gog")
        return G_if, G_og

    def banks_of(G_if, G_og):
        return [(G_if, 0, 0), (G_if, 1, 1), (G_og, 0, 2), (G_og, 1, 3)]

    def emit_seed(G_if, G_og):
        """K=1 bias seeds (general-bias path only); returns the set of
        seeded (started) bank keys."""
        seeded = set()
        if bias_const is not None:
            return seeded  # bias rides the ACT activation's scalar bias
        for Gp, n, g0 in banks_of(G_if, G_og):
            nc.tensor.matmul(Gp[:, n * H:(n + 1) * H], ones_bf[:, :],
                             b_bf[:, g0 * H:(g0 + 1) * H],
                             start=True, stop=False)
            seeded.add((id(Gp), n))
        return seeded

    def emit_terms(t, G_if, G_og, term_ids, started=None, bank_major=False,
                   banks=None):
        """3 fp8 terms x 4 DoubleRow k-blocks accumulate per bank; stop on the
        last term's last k-block. `started`: set of bank keys already started
        (by a seed or an earlier emit_terms call); None = none started."""
        ts_ = slice(t * P, (t + 1) * P)
        if banks is None:
            banks = banks_of(G_if, G_og)
        loops = ([(b, ti, j) for b in banks
                  for ti in term_ids if ti in GATE_TERMS[b[2]]
                  for j in range(DRB)]
                 if bank_major else
                 [(b, ti, j) for ti in term_ids for j in range(DRB)
                  for b in banks if ti in GATE_TERMS[b[2]]])
        started = set() if started is None else started
        for (Gp, n, g0), ti, j in loops:
            zo, wsb = TERMS[ti]
            key = (id(Gp), n)
            st = key not in started
            started.add(key)
            nc.tensor.matmul(
                Gp[:, n * H:(n + 1) * H],
                zhl_sb[:, zo + 2 * j:zo + 2 * j + 2, ts_],
                w_slice(wsb, j, g0),
                perf_mode=DR,
                start=st,
                stop=(ti == GATE_TERMS[g0][-1] and j == DRB - 1))
        return started

    def emit_bias_adds(G_if, G_og):
        if bias_const is not None:
            return  # bias rides the ACT activation's scalar bias
        nc.vector.tensor_add(G_if[:], G_if[:], b_bc[:, 0:2 * H])
        nc.vector.tensor_add(G_og[:], G_og[:], b_bc[:, 2 * H:4 * H])

    def emit_gates(t):
        # Bias: PE K=1 seed (256*b) for the last tile (shortest tail chain),
        # DVE post-add in steady state where PE cycles are scarce.
        bias_pe = t == NT - 1
        G_if, G_og = alloc_tile(t)
        started = emit_seed(G_if, G_og) if bias_pe else None
        if t == NT - 1:
            # bank-major in order g,i,f,o: the deepest consumers of the tail
            # chain (tanh_g -> tmp -> c -> tanh_c) stop earliest; only the
            # shallow o-gated ops trail the final matmul.
            # i/f are 1-term (4 matmuls each): stopping them first puts their
            # sigmoids ~2.5us before PE-end; g's deep chain starts at -1.3us
            # and only the shallow o-gated ops trail the final matmul.
            banks = [(G_if, 0, 0), (G_if, 1, 1), (G_og, 1, 3), (G_og, 0, 2)]
            emit_terms(t, G_if, G_og, [0, 1, 2], started=started,
                       bank_major=True, banks=banks)
        else:
            emit_terms(t, G_if, G_og, [0, 1, 2], started=started)
        if not bias_pe:
            emit_bias_adds(G_if, G_og)
        return G_if, G_og

    def _store(d, sb, q, tq, t):
        if q == NT // QUAD - 1:  # last quad: per-tile stores, shorter tail
            nc.sync.dma_start(
                out=d[t * P:(t + 1) * P, :].rearrange("(n p) d -> p n d", p=P),
                in_=sb[:, tq:tq + 1, :])
        elif tq == QUAD - 1:
            nc.sync.dma_start(out=dram_quad(d, q), in_=sb[:])

    def emit_epi_front(t, G_if, G_og):
        """Gate activations + the c chain. ACT scale=1/SG descales the fp8
        gate accumulation for free; with a per-gate-constant bias it also
        rides the scalar bias operand. (tanh_g before sig_o: the c chain
        needs g sooner than h needs o.)"""
        q, tq = divmod(t, QUAD)
        c4_sb, h4_sb = out_tiles[q]
        sg = 1.0 / SG
        if_s = epi.tile([P, 2 * H], BF16, tag="if_s")
        g_t = epi.tile([P, H], BF16, tag="g_t")
        o_s = epi.tile([P, H], BF16, tag="o_s")
        tmp = epi.tile([P, H], BF16, tag="tmp")
        c1 = epi.tile([P, H], BF16, tag="c1")
        i_s, f_s = if_s[:, 0:H], if_s[:, H:2 * H]
        bi, bf_, bo, bg = bias_const if bias_const is not None else (0.0,) * 4
        if bias_const is not None and bi != bf_:
            nc.scalar.activation(f_s, G_if[:, H:2 * H], AF.Sigmoid,
                                 scale=sg, bias=bf_)
            nc.scalar.activation(i_s, G_if[:, 0:H], AF.Sigmoid,
                                 scale=sg, bias=bi)
        else:
            nc.scalar.activation(if_s[:], G_if[:], AF.Sigmoid,
                                 scale=sg, bias=bi)
        nc.scalar.activation(g_t[:], G_og[:, H:2 * H], AF.Tanh,
                             scale=sg, bias=bg)
        nc.scalar.activation(o_s[:], G_og[:, 0:H], AF.Sigmoid,
                             scale=sg, bias=bo)
        nc.vector.tensor_mul(tmp[:], i_s, g_t[:])
        nc.gpsimd.tensor_mul(c1[:], f_s, c_all[:, t, :])
        nc.vector.tensor_add(c4_sb[:, tq, :], c1[:], tmp[:])
        _store(co_d, c4_sb, q, tq, t)
        return (t, o_s)

    def emit_epi_back(pend):
        """tanh(c) + h chain, emitted one tile late: its data is long ready,
        so the in-order ACT queue never stalls waiting on the c round-trip."""
        t, o_s = pend
        q, tq = divmod(t, QUAD)
        c4_sb, h4_sb = out_tiles[q]
        tanh_c = epi.tile([P, H], BF16, tag="tanh_c")
        h_pre = epi.tile([P, H], BF16, tag="h_pre")
        mv = stat_pool.tile([P, 2], F32, tag="mv")
        nc.scalar.activation(tanh_c[:], c4_sb[:, tq, :], AF.Tanh)
        nc.vector.tensor_mul(h_pre[:], o_s[:], tanh_c[:])
        st = stat_pool.tile([P, 6], F32, tag="st")
        nc.vector.bn_stats(out=st[:], in_=h_pre[:])
        nc.vector.bn_aggr(out=mv[:], in_=st[:])
        inv, nms = _newton_inv(nc, mybir, stat_pool, magic, mv, "s_")
        # h = (h_pre - mu) * inv  (ln scale/shift applied host-side)
        nc.vector.tensor_scalar(h4_sb[:, tq, :], h_pre[:], inv[:], nms[:],
                                op0=OP.mult, op1=OP.add)
        _store(ho_d, h4_sb, q, tq, t)

    def emit_epi(t, G_if, G_og):
        emit_epi_back(emit_epi_front(t, G_if, G_og))

    # Tiles 0/1 interleave: seeds + terms 1-2 for both tiles first, so t1's
    # hi/lo@Wh work covers the wl k-pair arrival latency; term 3 (hi@Wl)
    # lands last for both. Dummy K=1 matmuls bridge the first-DMA latency so
    # the PE p-state ramp is warm when real data arrives.
    G0 = alloc_tile(0)
    G1 = alloc_tile(1)
    for _ in range(8):
        # Dummies bridge the DMA-bound head (b16+wh01+zh ~5.5us serial) so
        # the PE p-state ramp is warm when real data arrives.
        nc.tensor.matmul(G0[0][:, 0:H], ones_bf[:, :], warm[:],
                         start=True, stop=True, skip_group_check=True)
    s01 = emit_seed(*G0) | emit_seed(*G1)

    def t01_block(term, j):
        zo, wsb = TERMS[term]
        for t, G in ((0, G0), (1, G1)):
            for Gp, n, g0 in banks_of(*G):
                if term not in GATE_TERMS[g0]:
                    continue
                key = (id(Gp), n)
                st = key not in s01
                s01.add(key)
                nc.tensor.matmul(
                    Gp[:, n * H:(n + 1) * H],
                    zhl_sb[:, zo + 2 * j:zo + 2 * j + 2, t * P:(t + 1) * P],
                    w_slice(wsb, j, g0),
                    perf_mode=DR, start=st,
                    stop=(term == GATE_TERMS[g0][-1] and j == DRB - 1))

    def zero_dummy(n=4):
        # K=1 matmuls of ones.T @ zeros accumulate +0 into the live t0 i-bank:
        # harmless filler that keeps the PE busy (and its p-state ramp hot)
        # across a DMA-arrival wait. A PE idle gap resets the ramp and the
        # whole queued burst then runs at the lowest p-state.
        for _ in range(n):
            nc.tensor.matmul(G0[0][:, 0:H // 2], ones_bf[:, :],
                             warm[:, 0:H // 2],
                             start=False, stop=False, skip_group_check=True)

    # hi@Wh j-interleaved across t0/t1 (paced by wh arrivals); then t0 runs
    # its remaining terms to completion so its epilogue (which frees the
    # PSUM pair for t2) overlaps t1's remaining matmuls.
    def t_term(t, G, term):
        zo, wsb = TERMS[term]
        for j in range(DRB):
            for Gp, n, g0 in banks_of(*G):
                if term not in GATE_TERMS[g0]:
                    continue
                nc.tensor.matmul(
                    Gp[:, n * H:(n + 1) * H],
                    zhl_sb[:, zo + 2 * j:zo + 2 * j + 2, t * P:(t + 1) * P],
                    w_slice(wsb, j, g0),
                    perf_mode=DR, start=False,
                    stop=(term == GATE_TERMS[g0][-1] and j == DRB - 1))

    for j in range(DRB):
        t01_block(0, j)
    t_term(0, G0, 1)
    t_term(0, G0, 2)
    t_term(1, G1, 1)
    t_term(1, G1, 2)
    def emit_epi_tail(Ga, Gb):
        """Interleaved epilogues for the last two tiles: t15's g-activation
        jumps ahead of t14's tanh_c on the in-order ACT queue, and every
        subsequent ACT op is emitted in bank-stop order, compressing the
        post-matmul tail."""
        ta, tb = NT - 2, NT - 1
        q, tqa = divmod(ta, QUAD)
        tqb = tb % QUAD
        c4_sb, h4_sb = out_tiles[q]
        sg = 1.0 / SG
        bi, bf_, bo, bg = bias_const if bias_const is not None else (0.0,) * 4
        HH = H // 2
        Ga_if, Ga_og = Ga
        Gb_if, Gb_og = Gb

        def tile_set(sfx):
            return (epi.tile([P, 2 * H], BF16, tag="if_s", name="if" + sfx),
                    epi.tile([P, H], BF16, tag="g_t", name="g" + sfx),
                    epi.tile([P, H], BF16, tag="o_s", name="o" + sfx),
                    epi.tile([P, H], BF16, tag="tmp", name="tmp" + sfx),
                    epi.tile([P, H], BF16, tag="c1", name="c1" + sfx),
                    epi.tile([P, H], BF16, tag="tanh_c", name="tc" + sfx),
                    epi.tile([P, H], BF16, tag="h_pre", name="hp" + sfx),
                    stat_pool.tile([P, 2], F32, tag="mv", name="mv" + sfx))

        if_a, g_a, o_a, tmp_a, c1_a, tc_a, hp_a, mv_a = tile_set("a")
        if_b, g_b, o_b, tmp_b, c1_b, tc_b, hp_b, mv_b = tile_set("b")

        def store(d, sb, tq, t):
            nc.sync.dma_start(
                out=d[t * P:(t + 1) * P, :].rearrange("(n p) d -> p n d", p=P),
                in_=sb[:, tq:tq + 1, :])

        # t14 front: acts + c chain
        if bias_const is not None and bi != bf_:
            nc.scalar.activation(if_a[:, H:2 * H], Ga_if[:, H:2 * H],
                                 AF.Sigmoid, scale=sg, bias=bf_)
            nc.scalar.activation(if_a[:, 0:H], Ga_if[:, 0:H],
                                 AF.Sigmoid, scale=sg, bias=bi)
        else:
            nc.scalar.activation(if_a[:], Ga_if[:], AF.Sigmoid,
                                 scale=sg, bias=bi)
        nc.scalar.activation(g_a[:], Ga_og[:, H:2 * H], AF.Tanh,
                             scale=sg, bias=bg)
        nc.scalar.activation(o_a[:], Ga_og[:, 0:H], AF.Sigmoid,
                             scale=sg, bias=bo)
        nc.vector.tensor_mul(tmp_a[:], if_a[:, 0:H], g_a[:])
        nc.gpsimd.tensor_mul(c1_a[:], if_a[:, H:2 * H], c_all[:, ta, :])
        nc.vector.tensor_add(c4_sb[:, tqa, :], c1_a[:], tmp_a[:])
        store(co_d, c4_sb, tqa, ta)
        # t15's g: its bank stopped ~3.9us before PE end
        nc.scalar.activation(g_b[:], Gb_og[:, H:2 * H], AF.Tanh,
                             scale=sg, bias=bg)
        # t14 back: tanh_c + h chain
        nc.scalar.activation(tc_a[:], c4_sb[:, tqa, :], AF.Tanh)
        nc.vector.tensor_mul(hp_a[:], o_a[:], tc_a[:])
        st_a = stat_pool.tile([P, 6], F32, tag="st")
        nc.vector.bn_stats(out=st_a[:], in_=hp_a[:])
        nc.vector.bn_aggr(out=mv_a[:], in_=st_a[:])
        inv, nms = _newton_inv(nc, mybir, stat_pool, magic, mv_a, "sa_")
        nc.vector.tensor_scalar(h4_sb[:, tqa, :], hp_a[:], inv[:], nms[:],
                                op0=OP.mult, op1=OP.add)
        store(ho_d, h4_sb, tqa, ta)
        # t15 rest in bank-stop order i, f, o
        nc.scalar.activation(if_b[:, 0:H], Gb_if[:, 0:H], AF.Sigmoid,
                             scale=sg, bias=bi)
        nc.vector.tensor_mul(tmp_b[:], if_b[:, 0:H], g_b[:])
        nc.scalar.activation(if_b[:, H:2 * H], Gb_if[:, H:2 * H], AF.Sigmoid,
                             scale=sg, bias=bf_)
        nc.vector.tensor_mul(c1_b[:], if_b[:, H:2 * H], c_all[:, tb, :])
        nc.vector.tensor_add(c4_sb[:, tqb, 0:HH], c1_b[:, 0:HH], tmp_b[:, 0:HH])
        nc.vector.tensor_add(c4_sb[:, tqb, HH:H], c1_b[:, HH:H], tmp_b[:, HH:H])
        store(co_d, c4_sb, tqb, tb)
        nc.scalar.activation(o_b[:], Gb_og[:, 0:H], AF.Sigmoid,
                             scale=sg, bias=bo)
        st2 = stat_pool.tile([P, 2, 6], F32, tag="st2")
        for jj in (0, 1):
            hs = slice(jj * HH, (jj + 1) * HH)
            nc.scalar.activation(tc_b[:, hs], c4_sb[:, tqb, hs], AF.Tanh)
            nc.vector.tensor_mul(hp_b[:, hs], o_b[:, hs], tc_b[:, hs])
            nc.vector.bn_stats(out=st2[:, jj, :], in_=hp_b[:, hs])
        nc.vector.bn_aggr(out=mv_b[:], in_=st2[:])
        inv, nms = _newton_inv(nc, mybir, stat_pool, magic, mv_b, "sb_")
        nc.vector.tensor_scalar(h4_sb[:, tqb, :], hp_b[:], inv[:], nms[:],
                                op0=OP.mult, op1=OP.add)
        store(ho_d, h4_sb, tqb, tb)

    emit_epi(0, *G0)
    pend = emit_epi_front(1, *G1)
    for t in range(2, NT - 2):
        G = emit_gates(t)
        nxt = emit_epi_front(t, *G)
        emit_epi_back(pend)
        pend = nxt
    G14 = emit_gates(NT - 2)
    G15 = emit_gates(NT - 1)
    emit_epi_back(pend)
    emit_epi_tail(G14, G15)


def _build(bias_const=None):
    key = ("nc", bias_const)
    if key in _CACHE:
        return _CACHE[key]
    from contextlib import ExitStack
    import concourse.tile as tile
    from concourse import bacc

    nc = bacc.Bacc("TRN2", target_bir_lowering=False, debug=False)
    with tile.TileContext(nc) as tc:
        with ExitStack() as ctx:
            _emit(nc, tc, ctx, bias_const)
    nc.compile()
    _CACHE[key] = nc
    return nc


def _np_bf16():
    from ml_dtypes import bfloat16
    return bfloat16


def _np_f8():
    from ml_dtypes import float8_e4m3
    return float8_e4m3


def _host_prep_weights(W_i, W_h, b):
    """Stack, gate-permute i|f|g|o -> i|f|o|g, scale by SW, split hi/lo fp8."""
    key = (id(W_i), id(W_h), id(b))
    if _CACHE.get("w_key") == key:
        return _CACHE["w"]
    f8 = _np_f8()
    perm = np.r_[0:2 * H, 3 * H:4 * H, 2 * H:3 * H]
    Wz = np.ascontiguousarray(
        np.vstack([np.asarray(W_i, np.float32), np.asarray(W_h, np.float32)])[:, perm]
    ) * np.float32(SW)
    Wh8 = Wz.astype(f8)
    Wl8 = (Wz - Wh8.astype(np.float32)).astype(f8)
    b_p = np.ascontiguousarray(np.asarray(b, np.float32)[perm])  # unscaled
    _CACHE["w"] = (Wh8, Wl8, b_p)
    _CACHE["w_key"] = key
    return _CACHE["w"]


def kernel(x, h_prev, c_prev, W_i, W_h, b, ln_weight, ln_bias):
    from concourse.bass_utils import run_bass_kernel_spmd

    bf16 = _np_bf16()
    f8 = _np_f8()
    Wh8, Wl8, b_p = _host_prep_weights(W_i, W_h, b)
    # Per-gate-constant bias (true for this module: b_f=1, rest 0) rides the
    # ACT activation's scalar bias; otherwise fall back to on-device adds.
    gate_slices = [b_p[g * H:(g + 1) * H] for g in range(4)]
    if all(np.all(v == v[0]) for v in gate_slices):
        bias_const = tuple(float(v[0]) for v in gate_slices)
    else:
        bias_const = None
    nc = _build(bias_const)
    lnw = np.asarray(ln_weight, np.float32)
    lnb = np.asarray(ln_bias, np.float32)
    x = np.asarray(x, np.float32)
    h_prev = np.asarray(h_prev, np.float32)
    c_prev = np.asarray(c_prev, np.float32)

    in_maps = []
    for c in range(N_CORES):
        rows = slice(c * BS, (c + 1) * BS)
        zT = np.ascontiguousarray(
            np.hstack([x[rows], h_prev[rows]]).T) * np.float32(SZ)
        zh8 = zT.astype(f8)
        zl8 = (zT - zh8.astype(np.float32)).astype(f8)
        im = {
            "zhl": np.ascontiguousarray(np.vstack([zh8, zl8])),
            "wh": Wh8,
            "wl": np.ascontiguousarray(Wl8[:, 2 * H:]),
            "c_prev": np.ascontiguousarray(c_prev[rows]).astype(bf16),
        }
        if bias_const is None:
            im["b16"] = (b_p * np.float32(SG)).astype(bf16)
        in_maps.append(im)
    res = run_bass_kernel_spmd(nc, in_maps, list(range(N_CORES)),
                               trace=bool(os.environ.get("KERNEL_TRACE")))
    _CACHE["last_res"] = res
    h = np.concatenate(
        [res.results[c]["h_out"] for c in range(N_CORES)], axis=0).astype(np.float32)
    c_out = np.concatenate(
        [res.results[c]["c_out"] for c in range(N_CORES)], axis=0).astype(np.float32)
    # ln affine: identity (ones/zeros) in this module's init; apply only if not
    if not (np.all(lnw == 1.0) and np.all(lnb == 0.0)):
        h = h * lnw + lnb
    return h, c_out
